# revision 23
# baseline (speedup 1.0000x reference)
"""Trainium2 Bass kernel: causal multi-head attention block (B=2, T=2048, C=1024, H=16).

Sharding: 8 cores = 2 (batch) x 4 (head groups of 4 heads).  Each core computes
q/k/v projections for its 4 heads, causal attention, and a partial out-proj
(rows of wo for its head slice).  Host sums the 4 partials per batch element.

v2: single software-pipelined schedule.  Attention for key-strip ks starts as
soon as strip-0 projections land; projection matmuls for later strips and
out-projection matmuls for earlier strips are interleaved between attention
j-steps so the tensor engine is never gated on the scalar engine's exp stream.

Math notes:
  - scores scale 1/sqrt(64) folded into wq/bq on the host.
  - softmax without max-subtraction (scores are O(+-10) here; exp safe in fp32).
  - softmax denominator via a ones-column appended to v (row 64 of the AV PSUM).
  - normalization: reciprocal of the denominator row, broadcast across
    partitions with a tiny K=2 f32r matmul (selector outer product), then one
    elementwise multiply.  No DRAM round-trips.
  - causal masking: post-exp affine_select (upper-triangle keep) on gpsimd.

Per-core layouts:
  xT      [1024, 2048]  x[b].T                          (bf16)
  wqkvT   [1024, 768]   [wq_s.T/8 | wk_s.T | wv_s.T]    (bf16)
  woT     [256, 1024]   wo[:, head_slice].T             (bf16)
  bqk     [128, 4]      cols: bq/8 (pair0,pair1), bk (pair0,pair1)
  bv_row  [1, 256], bo_row [1, 1024]
  y       [2048, 1024]  partial output (pre-sum)        (f32)
"""

import os
import sys

import numpy as np

try:
    import ml_dtypes
    BF16_NP = ml_dtypes.bfloat16
except ImportError:  # pragma: no cover
    BF16_NP = None

for _p in ("/opt/trn_rl_repo", "/root/.axon_site/_ro/trn_rl_repo"):
    if os.path.isdir(_p) and _p not in sys.path:
        sys.path.append(_p)

import concourse.bass as bass  # noqa: E402
import concourse.mybir as mybir  # noqa: E402
import concourse.tile as tile  # noqa: E402

F32 = mybir.dt.float32
F32R = mybir.dt.float32r
BF16 = mybir.dt.bfloat16

B, T, C, H = 2, 2048, 1024, 16
D = C // H          # 64
HPC = 4             # heads per core
DPC = HPC * D       # 256 head-dims per core
NCORES = 8

CHUNK = 128         # s-chunk / contraction granularity
SST = 256           # attention t-strip (PSUM-friendly)
PST = 512           # projection t-strip
NKS = T // SST      # 8 attention strips
NPS = T // PST      # 4 projection strips
VW = D + 1          # 65: v columns + ones column per head

_CTRL_TYPES = (mybir.InstDrain, mybir.InstNoOp, mybir.InstEventSemaphore)


def split_excess_waits(nc, lim=1):
    """Walrus accepts at most one sync-wait per instruction; move extras onto
    same-engine NoOps inserted just before the owner."""
    k = 0
    for fn in nc.m.functions:
        for blk in fn.blocks:
            out = []
            changed = False
            for inst in blk.instructions:
                si = inst.sync_info
                if si is not None and si.on_wait and len(si.on_wait) > lim:
                    waits = list(si.on_wait)
                    extra, keep = waits[:-lim], waits[-lim:]
                    for w in extra:
                        nop = mybir.InstNoOp(name=f"waitfix_{k}", ins=[], outs=[])
                        k += 1
                        nop.engine = inst.engine
                        nop.sync_info = mybir.SyncInfo(on_wait=[w], on_update=[])
                        out.append(nop)
                    si.on_wait = keep
                    changed = True
                out.append(inst)
            if changed:
                blk.instructions = out
    return k


def build_nc(t_len=T, fix_waits=True, debug_dump=False):
    """Build the per-core SPMD Bass program (same program on all 8 cores)."""
    assert t_len % PST == 0
    nks = t_len // SST
    nps = t_len // PST
    n_cchunk = C // CHUNK             # 8
    n_ttile = t_len // CHUNK          # 16

    nc = bass.Bass(target_bir_lowering=False)

    xT = nc.dram_tensor("xT", [C, t_len], BF16, kind="ExternalInput")
    wqkvT = nc.dram_tensor("wqkvT", [C, 3 * DPC], BF16, kind="ExternalInput")
    woT = nc.dram_tensor("woT", [DPC, C], BF16, kind="ExternalInput")
    bqk = nc.dram_tensor("bqk", [CHUNK, 4], F32, kind="ExternalInput")
    bv_row = nc.dram_tensor("bv_row", [1, DPC], F32, kind="ExternalInput")
    bo_row = nc.dram_tensor("bo_row", [1, C], F32, kind="ExternalInput")
    ones_row = nc.dram_tensor("ones_row", [1, CHUNK], F32R, kind="ExternalInput")
    y = nc.dram_tensor("y", [t_len, C], F32, kind="ExternalOutput")
    if debug_dump:
        dbg_av = nc.dram_tensor("dbg_av", [CHUNK, 2 * SST], F32, kind="ExternalOutput")
        dbg_e = nc.dram_tensor("dbg_e", [2, CHUNK, 2 * SST], F32, kind="ExternalOutput")

    Exp = mybir.ActivationFunctionType.Exp

    with tile.TileContext(nc) as tc:
        with tc.tile_pool(name="persist", bufs=1) as pp, \
             tc.tile_pool(name="work", bufs=1) as wp, \
             tc.tile_pool(name="ps", bufs=1, space="PSUM") as ps:
            # ---- constants / ones row for the normalize broadcast ----
            onecol = pp.tile([1, CHUNK], F32R, tag="onecol", name="onecol")
            nc.gpsimd.dma_start(out=onecol, in_=ones_row[:, :])

            # scalar-engine warmup: trigger the exp table load early
            warm = pp.tile([1, 8], F32, tag="warm", name="warm")
            nc.gpsimd.memset(warm, 0.0)
            nc.scalar.activation(warm, warm, Exp)

            # ---- input DMAs (spread across idle engine queues) ----
            bqk_sb = pp.tile([CHUNK, 4], F32, tag="bqk", name="bqk_sb")
            bv_bc = pp.tile([CHUNK, DPC], F32, tag="bv_bc", name="bv_bc")
            bo_bc = pp.tile([CHUNK, C], F32, tag="bo_bc", name="bo_bc")

            # xt strips: tags per c-chunk, two strip buffers each
            def load_xt(strip, eng):
                tiles = []
                for c in range(n_cchunk):
                    x_ = wp.tile([CHUNK, PST], BF16, tag=f"xt{c}", bufs=2,
                                 name=f"xt{c}_{strip}")
                    eng.dma_start(
                        out=x_,
                        in_=xT[c * CHUNK:(c + 1) * CHUNK,
                               strip * PST:(strip + 1) * PST])
                    tiles.append(x_)
                return tiles

            w_sb = []
            xt_cur = load_xt(0, nc.scalar)         # strip 0 on scalar queue
            for c in range(n_cchunk):
                w = pp.tile([CHUNK, 3 * DPC], BF16, tag=f"w{c}", name=f"w{c}")
                nc.gpsimd.dma_start(out=w, in_=wqkvT[c * CHUNK:(c + 1) * CHUNK, :])
                w_sb.append(w)
            xt_nxt = load_xt(1, nc.sync) if nps > 1 else None

            wo_sb = []
            for i in range(2):
                w = pp.tile([CHUNK, C], BF16, tag=f"wo{i}", name=f"wo{i}")
                nc.gpsimd.dma_start(out=w, in_=woT[i * CHUNK:(i + 1) * CHUNK, :])
                wo_sb.append(w)
            nc.gpsimd.dma_start(out=bqk_sb, in_=bqk[:, :])
            nc.gpsimd.dma_start(out=bv_bc, in_=bv_row[0:1, :].broadcast_to((CHUNK, DPC)))
            nc.gpsimd.dma_start(out=bo_bc, in_=bo_row[0:1, :].broadcast_to((CHUNK, C)))

            # ---- persistent activations ----
            # per-head q/k, transposed layout [d=64, t]
            qTh = [pp.tile([D, t_len], BF16, tag=f"qT{h}", name=f"qT{h}")
                   for h in range(4)]
            kTh = [pp.tile([D, t_len], BF16, tag=f"kT{h}", name=f"kT{h}")
                   for h in range(4)]
            # v tiles: [s-chunk 128, 4 heads x (64 v | ones)]
            vaug = [pp.tile([CHUNK, HPC * VW], BF16, tag=f"v{j}", name=f"v{j}")
                    for j in range(n_ttile)]
            for j in range(n_ttile):
                nc.vector.memset(vaug[j], 1.0)   # ones col survives; rest overwritten
            aoT = [pp.tile([CHUNK, t_len], BF16, tag=f"aoT{p}", name=f"aoT{p}")
                   for p in range(2)]

            # =========== emit helpers ===========
            def proj_qk_group(strip, pj, m, xt):
                """q (pj=0) or k (pj=1) projection for head-pair m over one
                512-wide t-strip."""
                pq = ps.tile([CHUNK, PST], F32, tag="fill", bufs=2,
                             name=f"pq{pj}{m}_{strip}")
                for c in range(n_cchunk):
                    nc.tensor.matmul(
                        pq,
                        w_sb[c][:, pj * DPC + m * CHUNK:pj * DPC + (m + 1) * CHUNK],
                        xt[c],
                        start=(c == 0), stop=(c == n_cchunk - 1))
                dst = kTh if pj else qTh
                eng0 = nc.vector          # gpsimd cannot read PSUM
                for hf in range(2):
                    lo, hi = hf * D, (hf + 1) * D
                    eng0.tensor_scalar_add(
                        dst[2 * m + hf][:, strip * PST:(strip + 1) * PST],
                        pq[lo:hi, :],
                        bqk_sb[lo:hi, 2 * pj + m:2 * pj + m + 1])

            def proj_v_group(strip, u, xt):
                """v projection for t-chunk jt = 4*strip + u."""
                jt = 4 * strip + u
                pv = ps.tile([CHUNK, PST], F32, tag="fill", bufs=2,
                             name=f"pv_{jt}")
                for c in range(n_cchunk):
                    nc.tensor.matmul(
                        pv[:, 0:DPC],
                        xt[c][:, u * CHUNK:(u + 1) * CHUNK],
                        w_sb[c][:, 2 * DPC:3 * DPC],
                        start=(c == 0), stop=(c == n_cchunk - 1))
                nc.vector.tensor_add(
                    vaug[jt].rearrange("p (h e) -> p h e", e=VW)[:, :, 0:D],
                    pv[:, 0:DPC].rearrange("p (h d) -> p h d", d=D),
                    bv_bc.rearrange("p (h d) -> p h d", d=D))

            def outproj_group(jt):
                """out-projection + bias + store for t-chunk jt."""
                for js in range(2):
                    py = ps.tile([CHUNK, PST], F32, tag="fill", bufs=2,
                                 name=f"py_{jt}_{js}")
                    for p in range(2):
                        nc.tensor.matmul(
                            py,
                            aoT[p][:, jt * CHUNK:(jt + 1) * CHUNK],
                            wo_sb[p][:, js * PST:(js + 1) * PST],
                            start=(p == 0), stop=(p == 1))
                    ysb = wp.tile([CHUNK, PST], F32, tag="ysb", bufs=2,
                                  name=f"ysb_{jt}_{js}")
                    nc.vector.tensor_add(
                        ysb, py, bo_bc[:, js * PST:(js + 1) * PST])
                    nc.sync.dma_start(
                        out=y[jt * CHUNK:(jt + 1) * CHUNK,
                              js * PST:(js + 1) * PST],
                        in_=ysb)

            def jstep(p, ks, j, nj):
                """scores -> exp -> (mask) -> AV for one s-chunk j of strip ks."""
                hA, hB = 2 * p, 2 * p + 1
                off = max(0, CHUNK * j - SST * ks)
                L = SST - off
                t0 = SST * ks + off
                sAB = ps.tile([CHUNK, 2 * SST], F32, tag="sAB", bufs=3,
                              name=f"s_{p}_{ks}_{j}")
                nc.tensor.matmul(
                    sAB[:, 0:L],
                    kTh[hA][:, j * CHUNK:(j + 1) * CHUNK],
                    qTh[hA][:, t0:t0 + L],
                    start=True, stop=True)
                nc.tensor.matmul(
                    sAB[:, L:2 * L],
                    kTh[hB][:, j * CHUNK:(j + 1) * CHUNK],
                    qTh[hB][:, t0:t0 + L],
                    start=True, stop=True)
                eAB = wp.tile([CHUNK, 2 * SST], BF16, tag="eAB", bufs=3,
                              name=f"e_{p}_{ks}_{j}")
                nc.scalar.activation(eAB[:, 0:2 * L], sAB[:, 0:2 * L], Exp)
                if debug_dump and p == 0 and ks == 0:
                    ecp = wp.tile([CHUNK, 2 * SST], F32, tag=f"ecp{j}",
                                  name=f"ecp_{j}")
                    nc.vector.memset(ecp, 0.0)
                    nc.vector.tensor_copy(ecp[:, 0:2 * L], eAB[:, 0:2 * L])
                    nc.sync.dma_start(out=dbg_e[j, :, :], in_=ecp)
                if CHUNK * j >= SST * ks:  # diagonal tile: zero upper triangle
                    for base in (0, L):
                        nc.gpsimd.affine_select(
                            out=eAB[:, base:base + CHUNK],
                            in_=eAB[:, base:base + CHUNK],
                            pattern=[[1, CHUNK]], channel_multiplier=-1,
                            base=0, compare_op=mybir.AluOpType.is_ge, fill=0.0)
                av = av_cur[0]
                nc.tensor.matmul(
                    av[0:VW, off:SST],
                    vaug[j][:, hA * VW:(hA + 1) * VW],
                    eAB[:, 0:L],
                    start=(j == 0), stop=(j == nj - 1), skip_group_check=True)
                # NOTE: start only on head A's j=0 matmul — a start=True marks
                # the whole 2KB PSUM zero-region pending-zero, so a second start
                # (head B) would reset head A's accumulation.  B's first write
                # lands on pending-zero bytes and overwrites, which is correct.
                nc.tensor.matmul(
                    av[0:VW, SST + off:2 * SST],
                    vaug[j][:, hB * VW:(hB + 1) * VW],
                    eAB[:, L:2 * L],
                    start=False, stop=(j == nj - 1), skip_group_check=True)

            def normalize(p, ks):
                """softmax-normalize strip ks of pair p into aoT[p]."""
                av = av_cur[0]
                if debug_dump and p == 0 and ks == 0:
                    avc = wp.tile([CHUNK, 2 * SST], F32, tag="avc", name="avc")
                    nc.vector.memset(avc, 0.0)
                    nc.vector.tensor_copy(avc[0:VW, :], av[0:VW, :])
                    nc.sync.dma_start(out=dbg_av[:, :], in_=avc)
                rr = wp.tile([1, 2 * SST], F32R, tag="rr", bufs=2,
                             name=f"rr_{p}_{ks}")
                with nc.allow_low_precision(reason="f32r == f32 bits; PE rounding only"):
                    nc.vector.reciprocal(rr[0:1, 0:SST], av[D:D + 1, 0:SST])
                    nc.vector.reciprocal(rr[0:1, SST:2 * SST],
                                         av[D:D + 1, SST:2 * SST])
                bcp = ps.tile([D, 2 * SST], F32, tag="bcf", bufs=1,
                              name=f"bcp_{p}_{ks}")
                nc.tensor.matmul(bcp[:, 0:SST], onecol[0:1, 0:D],
                                 rr[0:1, 0:SST],
                                 start=True, stop=True)
                nc.tensor.matmul(bcp[:, SST:2 * SST], onecol[0:1, 0:D],
                                 rr[0:1, SST:2 * SST],
                                 start=True, stop=True)
                bcf = wp.tile([CHUNK, SST], F32, tag="bcf_sb", bufs=2,
                              name=f"bcf_{p}_{ks}")
                nc.vector.tensor_copy(bcf[0:D, :], bcp[:, 0:SST])
                nc.vector.tensor_copy(bcf[D:CHUNK, :], bcp[:, SST:2 * SST])
                dstA = aoT[p][0:D, ks * SST:(ks + 1) * SST]
                nc.vector.tensor_mul(dstA, av[0:D, 0:SST], bcf[0:D, :])
                dstB = aoT[p][D:CHUNK, ks * SST:(ks + 1) * SST]
                nc.vector.tensor_copy(dstB, av[0:D, SST:2 * SST])
                nc.gpsimd.tensor_mul(dstB, dstB, bcf[D:CHUNK, :])

            # =========== schedule ===========
            # pre-loop: full projection of strip 0
            for pj in (1, 0):
                for m in range(2):
                    proj_qk_group(0, pj, m, xt_cur)
            for u in range(4):
                proj_v_group(0, u, xt_cur)

            av_cur = [None]
            for ks in range(nks):
                # filler groups for this ks
                fillers = []
                pstrip = ks // 2 + 1          # projection strip being prepared
                if pstrip < nps:
                    if ks % 2 == 0:           # k/q groups on even ks
                        xt = xt_nxt
                        for pj in (1, 0):
                            for m in range(2):
                                fillers.append(
                                    (proj_qk_group, (pstrip, pj, m, xt)))
                    else:                     # v groups on odd ks
                        xt = xt_nxt
                        for u in range(4):
                            fillers.append((proj_v_group, (pstrip, u, xt)))
                if ks >= 1:                   # out-proj of finished strips
                    for jt in (2 * (ks - 1), 2 * (ks - 1) + 1):
                        fillers.append((outproj_group, (jt,)))

                nj = 2 * ks + 2
                total_js = 2 * nj
                gi = 0
                cnt = 0
                for p in (0, 1):
                    av_cur[0] = ps.tile([CHUNK, 2 * SST], F32, tag="av", bufs=2,
                                        name=f"av_{p}_{ks}")
                    for j in range(nj):
                        jstep(p, ks, j, nj)
                        cnt += 1
                        want = (cnt * len(fillers)) // total_js
                        while gi < want:
                            fn, args = fillers[gi]
                            fn(*args)
                            gi += 1
                    normalize(p, ks)
                while gi < len(fillers):
                    fn, args = fillers[gi]
                    fn(*args)
                    gi += 1

                # rotate xt strip buffers / issue next strip loads
                if ks % 2 == 1 and pstrip < nps:
                    xt_cur = xt_nxt
                    if pstrip + 1 < nps:
                        xt_nxt = load_xt(pstrip + 1, nc.sync)

            # tail: last two out-proj chunks
            for jt in (2 * (nks - 1), 2 * (nks - 1) + 1):
                outproj_group(jt)

    if fix_waits:
        split_excess_waits(nc)
    return nc


def make_in_maps(x, wq, bq, wk, bk, wv, bv, wo, bo, t_len=T):
    """Build the 8 per-core input dicts from full inputs."""
    in_maps = []
    scale = 1.0 / np.sqrt(np.float32(D))
    for core in range(NCORES):
        b, hg = core // 4, core % 4
        sl = slice(DPC * hg, DPC * (hg + 1))
        wqs = (wq[sl] * scale).astype(np.float32)
        bqs = (bq[sl] * scale).astype(np.float32)
        wqkvT = np.concatenate([wqs.T, wk[sl].T, wv[sl].T], axis=1)
        bqk = np.stack([bqs[0:CHUNK], bqs[CHUNK:2 * CHUNK],
                        bk[sl][0:CHUNK], bk[sl][CHUNK:2 * CHUNK]], axis=1)
        bo_part = bo if hg == 0 else np.zeros_like(bo)
        in_maps.append({
            "xT": np.ascontiguousarray(x[b, :t_len].T).astype(BF16_NP),
            "wqkvT": np.ascontiguousarray(wqkvT).astype(BF16_NP),
            "woT": np.ascontiguousarray(wo[:, sl].T).astype(BF16_NP),
            "bqk": np.ascontiguousarray(bqk, dtype=np.float32),
            "bv_row": np.ascontiguousarray(bv[sl][None, :], dtype=np.float32),
            "bo_row": np.ascontiguousarray(bo_part[None, :], dtype=np.float32),
            "ones_row": np.ones((1, CHUNK), dtype=np.float32),
        })
    return in_maps


def gather_output(results, t_len=T):
    ys = [results[i]["y"] for i in range(NCORES)]
    return np.stack([ys[0] + ys[1] + ys[2] + ys[3],
                     ys[4] + ys[5] + ys[6] + ys[7]]).astype(np.float32)


_NC_CACHE = {}


def _get_nc(t_len=T):
    if t_len not in _NC_CACHE:
        _NC_CACHE[t_len] = build_nc(t_len)
    return _NC_CACHE[t_len]


def kernel(x, wq, bq, wk, bk, wv, bv, wo, bo, mask=None, **_unused):
    """Full-input entry point: shard, run on 8 NeuronCores, gather."""
    from concourse.bass_utils import run_bass_kernel_spmd

    x = np.asarray(x, dtype=np.float32)
    in_maps = make_in_maps(x, np.asarray(wq, np.float32), np.asarray(bq, np.float32),
                           np.asarray(wk, np.float32), np.asarray(bk, np.float32),
                           np.asarray(wv, np.float32), np.asarray(bv, np.float32),
                           np.asarray(wo, np.float32), np.asarray(bo, np.float32))
    nc = _get_nc(T)
    res = run_bass_kernel_spmd(nc, in_maps, list(range(NCORES)))
    return gather_output(res.results)


# revision 26
# speedup vs baseline: 1.6543x; 1.6543x over previous
"""Trainium2 Bass kernel: causal multi-head attention block (B=2, T=2048, C=1024, H=16).

Sharding: 8 cores = 2 (batch) x 4 (head groups of 4 heads).  Each core computes
q/k/v projections for its 4 heads, causal attention, and a partial out-proj
(rows of wo for its head slice).  Host sums the 4 partials per batch element.

v3: single software-pipelined schedule, tuned for the HAM throttle (PE runs at
2.4 GHz only while "warm"; activity gaps drop it to 1.2 GHz).  Projection
matmuls for strip P+1 are interleaved into attention strip P's j-steps, and the
out-projection is deferred into the last attention strip so every phase keeps
the tensor engine near-fully busy.

Math notes:
  - scores scale 1/sqrt(64) folded into wq/bq on the host.
  - softmax without max-subtraction (scores are O(+-10) here; exp safe in fp32).
  - softmax denominator via a ones-column appended to v (row 64 of the AV PSUM).
  - normalization: copy denominator row out, broadcast it across partitions
    with an SBUF->SBUF DMA, one reciprocal_approx_fast, one in-place multiply.
  - causal masking: post-exp affine_select (upper-triangle keep) on gpsimd.
  - scores matmuls run K=64 (per-head q/k tiles); head A and head B of a pair
    occupy separate PSUM banks so both accumulations can use start=True.

Per-core layouts:
  xT      [1024, 2048]  x[b].T                          (bf16)
  wqkvT   [1024, 768]   [wq_s.T/8 | wk_s.T | wv_s.T]    (bf16)
  woT     [256, 1024]   wo[:, head_slice].T             (bf16)
  bqk     [128, 4]      cols: bq/8 (pair0,pair1), bk (pair0,pair1)
  bv_row  [1, 256], bo_row [1, 1024]
  y       [2048, 1024]  partial output (pre-sum)        (f32)
"""

import os
import sys

import numpy as np

try:
    import ml_dtypes
    BF16_NP = ml_dtypes.bfloat16
except ImportError:  # pragma: no cover
    BF16_NP = None

for _p in ("/opt/trn_rl_repo", "/root/.axon_site/_ro/trn_rl_repo"):
    if os.path.isdir(_p) and _p not in sys.path:
        sys.path.append(_p)

import concourse.bass as bass  # noqa: E402
import concourse.mybir as mybir  # noqa: E402
import concourse.tile as tile  # noqa: E402

F32 = mybir.dt.float32
BF16 = mybir.dt.bfloat16

B, T, C, H = 2, 2048, 1024, 16
D = C // H          # 64
HPC = 4             # heads per core
DPC = HPC * D       # 256 head-dims per core
NCORES = 8

CHUNK = 128         # s-chunk / contraction granularity
SST = 512           # attention t-strip == one PSUM bank of f32
VW = D + 1          # 65: v columns + ones column per head

_CTRL_TYPES = (mybir.InstDrain, mybir.InstNoOp, mybir.InstEventSemaphore)


def split_excess_waits(nc, lim=1):
    """Walrus accepts at most one sync-wait per instruction; move extras onto
    same-engine NoOps inserted just before the owner."""
    k = 0
    for fn in nc.m.functions:
        for blk in fn.blocks:
            out = []
            changed = False
            for inst in blk.instructions:
                si = inst.sync_info
                if si is not None and si.on_wait and len(si.on_wait) > lim:
                    waits = list(si.on_wait)
                    extra, keep = waits[:-lim], waits[-lim:]
                    for w in extra:
                        nop = mybir.InstNoOp(name=f"waitfix_{k}", ins=[], outs=[])
                        k += 1
                        nop.engine = inst.engine
                        nop.sync_info = mybir.SyncInfo(on_wait=[w], on_update=[])
                        out.append(nop)
                    si.on_wait = keep
                    changed = True
                out.append(inst)
            if changed:
                blk.instructions = out
    return k


def build_nc(t_len=T, fix_waits=True):
    """Build the per-core SPMD Bass program (same program on all 8 cores)."""
    assert t_len % SST == 0
    nks = t_len // SST                # 4 strips
    n_cchunk = C // CHUNK             # 8
    n_ttile = t_len // CHUNK          # 16

    nc = bass.Bass(target_bir_lowering=False)

    xT = nc.dram_tensor("xT", [C, t_len], BF16, kind="ExternalInput")
    wqkvT = nc.dram_tensor("wqkvT", [C, 3 * DPC], BF16, kind="ExternalInput")
    woT = nc.dram_tensor("woT", [DPC, C], BF16, kind="ExternalInput")
    bqk = nc.dram_tensor("bqk", [CHUNK, 4], F32, kind="ExternalInput")
    bv_row = nc.dram_tensor("bv_row", [1, DPC], F32, kind="ExternalInput")
    bo_row = nc.dram_tensor("bo_row", [1, C], F32, kind="ExternalInput")
    y = nc.dram_tensor("y", [t_len, C], F32, kind="ExternalOutput")

    Exp = mybir.ActivationFunctionType.Exp

    with tile.TileContext(nc) as tc:
        with tc.tile_pool(name="persist", bufs=1) as pp, \
             tc.tile_pool(name="work", bufs=1) as wp, \
             tc.tile_pool(name="dr", bufs=1, space="DRAM") as dr, \
             tc.tile_pool(name="ps", bufs=1, space="PSUM") as ps:
            # scalar-engine warmup: trigger the exp table load early
            warm = pp.tile([1, 8], F32, tag="warm", name="warm")
            nc.gpsimd.memset(warm, 0.0)
            nc.scalar.activation(warm, warm, Exp)

            # ---- input DMAs (spread across idle engine queues) ----
            bqk_sb = pp.tile([CHUNK, 4], F32, tag="bqk", name="bqk_sb")
            bv_bc = pp.tile([CHUNK, DPC], F32, tag="bv_bc", name="bv_bc")
            bo_bc = pp.tile([CHUNK, C], F32, tag="bo_bc", name="bo_bc")

            def load_xt(strip, eng):
                tiles = []
                for c in range(n_cchunk):
                    x_ = wp.tile([CHUNK, SST], BF16, tag=f"xt{c}", bufs=2,
                                 name=f"xt{c}_{strip}")
                    eng.dma_start(
                        out=x_,
                        in_=xT[c * CHUNK:(c + 1) * CHUNK,
                               strip * SST:(strip + 1) * SST])
                    tiles.append(x_)
                return tiles

            w_sb = []
            xt_bufs = {0: load_xt(0, nc.scalar)}      # strip 0 on scalar queue
            for c in range(n_cchunk):
                w = pp.tile([CHUNK, 3 * DPC], BF16, tag=f"w{c}", name=f"w{c}")
                nc.gpsimd.dma_start(out=w, in_=wqkvT[c * CHUNK:(c + 1) * CHUNK, :])
                w_sb.append(w)
            if nks > 1:
                xt_bufs[1] = load_xt(1, nc.sync)

            wo_sb = []
            for i in range(2):
                w = pp.tile([CHUNK, C], BF16, tag=f"wo{i}", name=f"wo{i}")
                nc.gpsimd.dma_start(out=w, in_=woT[i * CHUNK:(i + 1) * CHUNK, :])
                wo_sb.append(w)
            nc.gpsimd.dma_start(out=bqk_sb, in_=bqk[:, :])
            nc.gpsimd.dma_start(out=bv_bc, in_=bv_row[0:1, :].broadcast_to((CHUNK, DPC)))
            nc.gpsimd.dma_start(out=bo_bc, in_=bo_row[0:1, :].broadcast_to((CHUNK, C)))

            # ---- persistent activations ----
            qTh = [pp.tile([D, t_len], BF16, tag=f"qT{h}", name=f"qT{h}")
                   for h in range(4)]
            kTh = [pp.tile([D, t_len], BF16, tag=f"kT{h}", name=f"kT{h}")
                   for h in range(4)]
            vaug = [pp.tile([CHUNK, HPC * VW], BF16, tag=f"v{j}", name=f"v{j}")
                    for j in range(n_ttile)]
            for j in range(n_ttile):
                nc.vector.memset(vaug[j], 1.0)   # ones col survives; rest overwritten
            aoT = [pp.tile([CHUNK, t_len], BF16, tag=f"aoT{p}", name=f"aoT{p}")
                   for p in range(2)]

            # =========== emit helpers ===========
            def proj_qk_group(strip, pj, m, xt):
                pq = ps.tile([CHUNK, SST], F32, tag="fill", bufs=2,
                             name=f"pq{pj}{m}_{strip}")
                for c in range(n_cchunk):
                    nc.tensor.matmul(
                        pq,
                        w_sb[c][:, pj * DPC + m * CHUNK:pj * DPC + (m + 1) * CHUNK],
                        xt[c],
                        start=(c == 0), stop=(c == n_cchunk - 1))
                dst = kTh if pj else qTh
                for hf in range(2):
                    lo, hi = hf * D, (hf + 1) * D
                    nc.vector.tensor_scalar_add(
                        dst[2 * m + hf][:, strip * SST:(strip + 1) * SST],
                        pq[lo:hi, :],
                        bqk_sb[lo:hi, 2 * pj + m:2 * pj + m + 1])

            def proj_v_group(strip, u, xt):
                jt = 4 * strip + u
                pv = ps.tile([CHUNK, SST], F32, tag="fill", bufs=2,
                             name=f"pv_{jt}")
                for c in range(n_cchunk):
                    nc.tensor.matmul(
                        pv[:, 0:DPC],
                        xt[c][:, u * CHUNK:(u + 1) * CHUNK],
                        w_sb[c][:, 2 * DPC:3 * DPC],
                        start=(c == 0), stop=(c == n_cchunk - 1))
                nc.vector.tensor_add(
                    vaug[jt].rearrange("p (h e) -> p h e", e=VW)[:, :, 0:D],
                    pv[:, 0:DPC].rearrange("p (h d) -> p h d", d=D),
                    bv_bc.rearrange("p (h d) -> p h d", d=D))

            def outproj_group(jt):
                for js in range(2):
                    py = ps.tile([CHUNK, SST], F32, tag="fill", bufs=2,
                                 name=f"py_{jt}_{js}")
                    for p in range(2):
                        nc.tensor.matmul(
                            py,
                            aoT[p][:, jt * CHUNK:(jt + 1) * CHUNK],
                            wo_sb[p][:, js * SST:(js + 1) * SST],
                            start=(p == 0), stop=(p == 1))
                    ysb = wp.tile([CHUNK, SST], F32, tag="ysb", bufs=2,
                                  name=f"ysb_{jt}_{js}")
                    nc.vector.tensor_add(
                        ysb, py, bo_bc[:, js * SST:(js + 1) * SST])
                    nc.sync.dma_start(
                        out=y[jt * CHUNK:(jt + 1) * CHUNK,
                              js * SST:(js + 1) * SST],
                        in_=ysb)

            def jstep(p, ks, j, nj):
                """scores -> exp -> (mask) -> AV for one s-chunk j of strip ks.

                PSUM layout of sAB / av: head A in bank 0 (cols 0:512), head B
                in bank 1 (cols 512:1024) — each bank gets its own start=True.
                """
                hA, hB = 2 * p, 2 * p + 1
                off = max(0, CHUNK * j - SST * ks)
                L = SST - off
                t0 = SST * ks + off
                sAB = ps.tile([CHUNK, 2 * SST], F32, tag="sAB", bufs=2,
                              name=f"s_{p}_{ks}_{j}")
                nc.tensor.matmul(
                    sAB[:, 0:L],
                    kTh[hA][:, j * CHUNK:(j + 1) * CHUNK],
                    qTh[hA][:, t0:t0 + L],
                    start=True, stop=True)
                nc.tensor.matmul(
                    sAB[:, SST:SST + L],
                    kTh[hB][:, j * CHUNK:(j + 1) * CHUNK],
                    qTh[hB][:, t0:t0 + L],
                    start=True, stop=True)
                eAB = wp.tile([CHUNK, 2 * SST], BF16, tag="eAB", bufs=3,
                              name=f"e_{p}_{ks}_{j}")
                if L == SST:
                    nc.scalar.activation(eAB, sAB, Exp)
                else:
                    nc.scalar.activation(eAB[:, 0:L], sAB[:, 0:L], Exp)
                    nc.scalar.activation(eAB[:, SST:SST + L],
                                         sAB[:, SST:SST + L], Exp)
                if CHUNK * j >= SST * ks:  # diagonal tile: zero upper triangle
                    for base in (0, SST):
                        nc.gpsimd.affine_select(
                            out=eAB[:, base:base + CHUNK],
                            in_=eAB[:, base:base + CHUNK],
                            pattern=[[1, CHUNK]], channel_multiplier=-1,
                            base=0, compare_op=mybir.AluOpType.is_ge, fill=0.0)
                av = av_cur[0]
                nc.tensor.matmul(
                    av[0:VW, off:SST],
                    vaug[j][:, hA * VW:(hA + 1) * VW],
                    eAB[:, 0:L],
                    start=(j == 0), stop=(j == nj - 1), skip_group_check=True)
                nc.tensor.matmul(
                    av[0:VW, SST + off:2 * SST],
                    vaug[j][:, hB * VW:(hB + 1) * VW],
                    eAB[:, SST:SST + L],
                    start=(j == 0), stop=(j == nj - 1), skip_group_check=True)

            def normalize(p, ks):
                """softmax-normalize strip ks of pair p into aoT[p]."""
                av = av_cur[0]
                strip = slice(ks * SST, (ks + 1) * SST)
                den = wp.tile([1, 2 * SST], F32, tag="den", bufs=2,
                              name=f"den_{p}_{ks}")
                # evict (unnormalized) + denominator row; frees the av banks
                if ks < nks - 1:
                    nc.scalar.activation(den, av[D:D + 1, :],
                                         mybir.ActivationFunctionType.Copy)
                else:
                    nc.vector.tensor_copy(den, av[D:D + 1, :])
                nc.vector.tensor_copy(aoT[p][0:D, strip], av[0:D, 0:SST])
                nc.vector.tensor_copy(aoT[p][D:CHUNK, strip],
                                      av[0:D, SST:2 * SST])
                # reciprocal via a [128, 8] reshape (wide on the DVE lanes),
                # then partition-broadcast — both through small DRAM bounces
                # (SBUF APs cannot reshape across partitions / stride-0 bcast)
                dden = dr.tile([1, 2 * SST], F32, tag="dden", bufs=2,
                               name=f"dden_{p}_{ks}")
                nc.gpsimd.dma_start(out=dden, in_=den)
                nf = 2 * SST // CHUNK
                dsb = wp.tile([CHUNK, nf], F32, tag="dsb", bufs=2,
                              name=f"dsb_{p}_{ks}")
                dview = dden.rearrange("a b -> (a b)").rearrange(
                    "(p f) -> p f", p=CHUNK)
                nc.gpsimd.dma_start(out=dsb, in_=dview)
                rsb = wp.tile([CHUNK, nf], F32, tag="rsb", bufs=2,
                              name=f"rsb_{p}_{ks}")
                nc.vector.reciprocal(rsb, dsb)
                drec = dr.tile([1, 2 * SST], F32, tag="drec", bufs=2,
                               name=f"drec_{p}_{ks}")
                rview = drec.rearrange("a b -> (a b)").rearrange(
                    "(p f) -> p f", p=CHUNK)
                nc.gpsimd.dma_start(out=rview, in_=rsb)
                bcf = wp.tile([CHUNK, SST], F32, tag="bcf", bufs=2,
                              name=f"bcf_{p}_{ks}")
                nc.gpsimd.dma_start(
                    out=bcf[0:D, :],
                    in_=drec[0:1, 0:SST].broadcast_to((D, SST)))
                nc.gpsimd.dma_start(
                    out=bcf[D:CHUNK, :],
                    in_=drec[0:1, SST:2 * SST].broadcast_to((D, SST)))
                nc.gpsimd.tensor_mul(aoT[p][:, strip], aoT[p][:, strip], bcf)

            # =========== schedule ===========
            # pre-loop: full projection of strip 0
            for pj in (1, 0):
                for m in range(2):
                    proj_qk_group(0, pj, m, xt_bufs[0])
            for u in range(4):
                proj_v_group(0, u, xt_bufs[0])

            av_cur = [None]
            for ks in range(nks):
                # prefetch x strip ks+2 (its buffer was freed by proj(ks))
                if ks + 2 < nks:
                    xt_bufs[ks + 2] = load_xt(ks + 2, nc.sync)

                fillers = []
                if ks + 1 < nks:                  # projection of next strip
                    xt = xt_bufs[ks + 1]
                    for pj in (1, 0):
                        for m in range(2):
                            fillers.append((proj_qk_group, (ks + 1, pj, m, xt)))
                    for u in range(4):
                        fillers.append((proj_v_group, (ks + 1, u, xt)))
                if ks == nks - 1:                 # deferred out-proj (strips 0..ks-1)
                    for jt in range(4 * (nks - 1)):
                        fillers.append((outproj_group, (jt,)))

                nj = 4 * ks + 4
                total_js = 2 * nj
                gi = 0
                cnt = 0
                for p in (0, 1):
                    av_cur[0] = ps.tile([CHUNK, 2 * SST], F32, tag="av", bufs=1,
                                        name=f"av_{p}_{ks}")
                    for j in range(nj):
                        jstep(p, ks, j, nj)
                        cnt += 1
                        want = (cnt * len(fillers)) // total_js
                        while gi < want:
                            fn, args = fillers[gi]
                            fn(*args)
                            gi += 1
                    normalize(p, ks)
                    # keep the tensor queue fed while av drains
                    if gi < len(fillers):
                        fn, args = fillers[gi]
                        fn(*args)
                        gi += 1
                while gi < len(fillers):
                    fn, args = fillers[gi]
                    fn(*args)
                    gi += 1

            # tail: out-proj of the last strip
            for jt in range(4 * (nks - 1), 4 * nks):
                outproj_group(jt)

    if fix_waits:
        split_excess_waits(nc)
    return nc


def make_in_maps(x, wq, bq, wk, bk, wv, bv, wo, bo, t_len=T):
    """Build the 8 per-core input dicts from full inputs."""
    in_maps = []
    scale = 1.0 / np.sqrt(np.float32(D))
    for core in range(NCORES):
        b, hg = core // 4, core % 4
        sl = slice(DPC * hg, DPC * (hg + 1))
        wqs = (wq[sl] * scale).astype(np.float32)
        bqs = (bq[sl] * scale).astype(np.float32)
        wqkvT = np.concatenate([wqs.T, wk[sl].T, wv[sl].T], axis=1)
        bqk = np.stack([bqs[0:CHUNK], bqs[CHUNK:2 * CHUNK],
                        bk[sl][0:CHUNK], bk[sl][CHUNK:2 * CHUNK]], axis=1)
        bo_part = bo if hg == 0 else np.zeros_like(bo)
        in_maps.append({
            "xT": np.ascontiguousarray(x[b, :t_len].T).astype(BF16_NP),
            "wqkvT": np.ascontiguousarray(wqkvT).astype(BF16_NP),
            "woT": np.ascontiguousarray(wo[:, sl].T).astype(BF16_NP),
            "bqk": np.ascontiguousarray(bqk, dtype=np.float32),
            "bv_row": np.ascontiguousarray(bv[sl][None, :], dtype=np.float32),
            "bo_row": np.ascontiguousarray(bo_part[None, :], dtype=np.float32),
        })
    return in_maps


def gather_output(results, t_len=T):
    ys = [results[i]["y"] for i in range(NCORES)]
    return np.stack([ys[0] + ys[1] + ys[2] + ys[3],
                     ys[4] + ys[5] + ys[6] + ys[7]]).astype(np.float32)


_NC_CACHE = {}


def _get_nc(t_len=T):
    if t_len not in _NC_CACHE:
        _NC_CACHE[t_len] = build_nc(t_len)
    return _NC_CACHE[t_len]


def kernel(x, wq, bq, wk, bk, wv, bv, wo, bo, mask=None, **_unused):
    """Full-input entry point: shard, run on 8 NeuronCores, gather."""
    from concourse.bass_utils import run_bass_kernel_spmd

    x = np.asarray(x, dtype=np.float32)
    in_maps = make_in_maps(x, np.asarray(wq, np.float32), np.asarray(bq, np.float32),
                           np.asarray(wk, np.float32), np.asarray(bk, np.float32),
                           np.asarray(wv, np.float32), np.asarray(bv, np.float32),
                           np.asarray(wo, np.float32), np.asarray(bo, np.float32))
    nc = _get_nc(T)
    res = run_bass_kernel_spmd(nc, in_maps, list(range(NCORES)))
    return gather_output(res.results)


# revision 29
# speedup vs baseline: 1.7353x; 1.0490x over previous
"""Trainium2 Bass kernel: causal multi-head attention block (B=2, T=2048, C=1024, H=16).

Sharding: 8 cores = 2 (batch) x 4 (head groups of 4 heads).  Each core computes
q/k/v projections for its 4 heads, causal attention, and a partial out-proj
(rows of wo for its head slice).  Host sums the 4 partials per batch element.

v3: single software-pipelined schedule, tuned for the HAM throttle (PE runs at
2.4 GHz only while "warm"; activity gaps drop it to 1.2 GHz).  Projection
matmuls for strip P+1 are interleaved into attention strip P's j-steps, and the
out-projection is deferred into the last attention strip so every phase keeps
the tensor engine near-fully busy.

Math notes:
  - scores scale 1/sqrt(64) folded into wq/bq on the host.
  - softmax without max-subtraction (scores are O(+-10) here; exp safe in fp32).
  - softmax denominator via a ones-column appended to v (row 64 of the AV PSUM).
  - normalization: copy denominator row out, broadcast it across partitions
    with an SBUF->SBUF DMA, one reciprocal_approx_fast, one in-place multiply.
  - causal masking: post-exp affine_select (upper-triangle keep) on gpsimd.
  - scores matmuls run K=64 (per-head q/k tiles); head A and head B of a pair
    occupy separate PSUM banks so both accumulations can use start=True.

Per-core layouts:
  xT      [1024, 2048]  x[b].T                          (bf16)
  wqkvT   [1024, 768]   [wq_s.T/8 | wk_s.T | wv_s.T]    (bf16)
  woT     [256, 1024]   wo[:, head_slice].T             (bf16)
  bqk     [128, 4]      cols: bq/8 (pair0,pair1), bk (pair0,pair1)
  bv_row  [1, 256], bo_row [1, 1024]
  y       [2048, 1024]  partial output (pre-sum)        (f32)
"""

import os
import sys

import numpy as np

try:
    import ml_dtypes
    BF16_NP = ml_dtypes.bfloat16
except ImportError:  # pragma: no cover
    BF16_NP = None

for _p in ("/opt/trn_rl_repo", "/root/.axon_site/_ro/trn_rl_repo"):
    if os.path.isdir(_p) and _p not in sys.path:
        sys.path.append(_p)

import concourse.bass as bass  # noqa: E402
import concourse.mybir as mybir  # noqa: E402
import concourse.tile as tile  # noqa: E402

F32 = mybir.dt.float32
BF16 = mybir.dt.bfloat16

B, T, C, H = 2, 2048, 1024, 16
D = C // H          # 64
HPC = 4             # heads per core
DPC = HPC * D       # 256 head-dims per core
NCORES = 8

CHUNK = 128         # s-chunk / contraction granularity
SST = 512           # attention t-strip == one PSUM bank of f32
VW = D + 1          # 65: v columns + ones column per head

_CTRL_TYPES = (mybir.InstDrain, mybir.InstNoOp, mybir.InstEventSemaphore)


def split_excess_waits(nc, lim=1):
    """Walrus accepts at most one sync-wait per instruction; move extras onto
    same-engine NoOps inserted just before the owner."""
    k = 0
    for fn in nc.m.functions:
        for blk in fn.blocks:
            out = []
            changed = False
            for inst in blk.instructions:
                si = inst.sync_info
                if si is not None and si.on_wait and len(si.on_wait) > lim:
                    waits = list(si.on_wait)
                    extra, keep = waits[:-lim], waits[-lim:]
                    for w in extra:
                        nop = mybir.InstNoOp(name=f"waitfix_{k}", ins=[], outs=[])
                        k += 1
                        nop.engine = inst.engine
                        nop.sync_info = mybir.SyncInfo(on_wait=[w], on_update=[])
                        out.append(nop)
                    si.on_wait = keep
                    changed = True
                out.append(inst)
            if changed:
                blk.instructions = out
    return k


def build_nc(t_len=T, fix_waits=True):
    """Build the per-core SPMD Bass program (same program on all 8 cores)."""
    assert t_len % SST == 0
    nks = t_len // SST                # 4 strips
    n_cchunk = C // CHUNK             # 8
    n_ttile = t_len // CHUNK          # 16

    nc = bass.Bass(target_bir_lowering=False)

    xT = nc.dram_tensor("xT", [C, t_len], BF16, kind="ExternalInput")
    wqkvT = nc.dram_tensor("wqkvT", [C, 3 * DPC], BF16, kind="ExternalInput")
    woT = nc.dram_tensor("woT", [DPC, C], BF16, kind="ExternalInput")
    bqk = nc.dram_tensor("bqk", [CHUNK, 4], F32, kind="ExternalInput")
    bv_row = nc.dram_tensor("bv_row", [1, DPC], F32, kind="ExternalInput")
    y = nc.dram_tensor("y", [t_len, C], F32, kind="ExternalOutput")

    Exp = mybir.ActivationFunctionType.Exp

    with tile.TileContext(nc) as tc:
        with tc.tile_pool(name="persist", bufs=1) as pp, \
             tc.tile_pool(name="work", bufs=1) as wp, \
             tc.tile_pool(name="dr", bufs=1, space="DRAM") as dr, \
             tc.tile_pool(name="ps", bufs=1, space="PSUM") as ps:
            # scalar-engine warmup: trigger the exp table load early
            warm = pp.tile([1, 8], F32, tag="warm", name="warm")
            nc.gpsimd.memset(warm, 0.0)
            nc.scalar.activation(warm, warm, Exp)

            # ---- input DMAs (spread across idle engine queues) ----
            bqk_sb = pp.tile([CHUNK, 4], F32, tag="bqk", name="bqk_sb")
            bv_bc = pp.tile([CHUNK, DPC], F32, tag="bv_bc", name="bv_bc")

            def load_xt(strip, engs):
                tiles = []
                for c in range(n_cchunk):
                    x_ = wp.tile([CHUNK, SST], BF16, tag=f"xt{c}", bufs=2,
                                 name=f"xt{c}_{strip}")
                    engs[c % len(engs)].dma_start(
                        out=x_,
                        in_=xT[c * CHUNK:(c + 1) * CHUNK,
                               strip * SST:(strip + 1) * SST])
                    tiles.append(x_)
                return tiles

            # interleave weight-chunk and x-chunk triggers across three idle
            # queues so the first projection groups unblock in c order
            nc.gpsimd.dma_start(out=bqk_sb, in_=bqk[:, :])
            nc.gpsimd.dma_start(out=bv_bc, in_=bv_row[0:1, :].broadcast_to((CHUNK, DPC)))
            w_sb = []
            xt_bufs = {}
            xt0 = []
            for c in range(n_cchunk):
                w = pp.tile([CHUNK, 3 * DPC], BF16, tag=f"w{c}", name=f"w{c}")
                nc.gpsimd.dma_start(out=w, in_=wqkvT[c * CHUNK:(c + 1) * CHUNK, :])
                w_sb.append(w)
                x_ = wp.tile([CHUNK, SST], BF16, tag=f"xt{c}", bufs=2,
                             name=f"xt{c}_0")
                (nc.scalar if c % 2 == 0 else nc.sync).dma_start(
                    out=x_, in_=xT[c * CHUNK:(c + 1) * CHUNK, 0:SST])
                xt0.append(x_)
            xt_bufs[0] = xt0
            if nks > 1:
                xt_bufs[1] = load_xt(1, [nc.sync, nc.scalar])

            wo_sb = []
            for i in range(2):
                w = pp.tile([CHUNK, C], BF16, tag=f"wo{i}", name=f"wo{i}")
                nc.gpsimd.dma_start(out=w, in_=woT[i * CHUNK:(i + 1) * CHUNK, :])
                wo_sb.append(w)

            # ---- persistent activations ----
            qTh = [pp.tile([D, t_len], BF16, tag=f"qT{h}", name=f"qT{h}")
                   for h in range(4)]
            kTh = [pp.tile([D, t_len], BF16, tag=f"kT{h}", name=f"kT{h}")
                   for h in range(4)]
            vaug = [pp.tile([CHUNK, HPC * VW], BF16, tag=f"v{j}", name=f"v{j}")
                    for j in range(n_ttile)]
            for j in range(n_ttile):
                nc.gpsimd.memset(vaug[j], 1.0)   # ones col survives; rest overwritten
            aoT = [pp.tile([CHUNK, t_len], BF16, tag=f"aoT{p}", name=f"aoT{p}")
                   for p in range(2)]

            # =========== emit helpers ===========
            def proj_qk_group(strip, pj, m, xt):
                pq = ps.tile([CHUNK, SST], F32, tag="fill", bufs=2,
                             name=f"pq{pj}{m}_{strip}")
                for c in range(n_cchunk):
                    nc.tensor.matmul(
                        pq,
                        w_sb[c][:, pj * DPC + m * CHUNK:pj * DPC + (m + 1) * CHUNK],
                        xt[c],
                        start=(c == 0), stop=(c == n_cchunk - 1))
                dst = kTh if pj else qTh
                for hf in range(2):
                    lo, hi = hf * D, (hf + 1) * D
                    nc.vector.tensor_scalar_add(
                        dst[2 * m + hf][:, strip * SST:(strip + 1) * SST],
                        pq[lo:hi, :],
                        bqk_sb[lo:hi, 2 * pj + m:2 * pj + m + 1])

            def proj_v_group(strip, u, xt):
                jt = 4 * strip + u
                pv = ps.tile([CHUNK, SST], F32, tag="fill", bufs=2,
                             name=f"pv_{jt}")
                for c in range(n_cchunk):
                    nc.tensor.matmul(
                        pv[:, 0:DPC],
                        xt[c][:, u * CHUNK:(u + 1) * CHUNK],
                        w_sb[c][:, 2 * DPC:3 * DPC],
                        start=(c == 0), stop=(c == n_cchunk - 1))
                nc.vector.tensor_add(
                    vaug[jt].rearrange("p (h e) -> p h e", e=VW)[:, :, 0:D],
                    pv[:, 0:DPC].rearrange("p (h d) -> p h d", d=D),
                    bv_bc.rearrange("p (h d) -> p h d", d=D))

            def outproj_group(jt, tail=False):
                for js in range(2):
                    py = ps.tile([CHUNK, SST], F32, tag="fill", bufs=2,
                                 name=f"py_{jt}_{js}")
                    for p in range(2):
                        nc.tensor.matmul(
                            py,
                            aoT[p][:, jt * CHUNK:(jt + 1) * CHUNK],
                            wo_sb[p][:, js * SST:(js + 1) * SST],
                            start=(p == 0), stop=(p == 1))
                    ysb = wp.tile([CHUNK, SST], F32, tag="ysb", bufs=2,
                                  name=f"ysb_{jt}_{js}")
                    if tail:      # scalar engine is idle post-exp
                        nc.scalar.activation(ysb, py,
                                             mybir.ActivationFunctionType.Copy)
                    else:
                        nc.vector.tensor_copy(ysb, py)
                    eng = (nc.scalar if (tail and js == 1) else nc.sync)
                    eng.dma_start(
                        out=y[jt * CHUNK:(jt + 1) * CHUNK,
                              js * SST:(js + 1) * SST],
                        in_=ysb)

            def jstep(p, ks, j, nj):
                """scores -> exp -> (mask) -> AV for one s-chunk j of strip ks.

                PSUM layout of sAB / av: head A in bank 0 (cols 0:512), head B
                in bank 1 (cols 512:1024) — each bank gets its own start=True.
                """
                hA, hB = 2 * p, 2 * p + 1
                off = max(0, CHUNK * j - SST * ks)
                L = SST - off
                t0 = SST * ks + off
                sAB = ps.tile([CHUNK, 2 * SST], F32, tag="sAB", bufs=2,
                              name=f"s_{p}_{ks}_{j}")
                nc.tensor.matmul(
                    sAB[:, 0:L],
                    kTh[hA][:, j * CHUNK:(j + 1) * CHUNK],
                    qTh[hA][:, t0:t0 + L],
                    start=True, stop=True)
                nc.tensor.matmul(
                    sAB[:, SST:SST + L],
                    kTh[hB][:, j * CHUNK:(j + 1) * CHUNK],
                    qTh[hB][:, t0:t0 + L],
                    start=True, stop=True)
                eAB = wp.tile([CHUNK, 2 * SST], BF16, tag="eAB", bufs=3,
                              name=f"e_{p}_{ks}_{j}")
                if L == SST:
                    nc.scalar.activation(eAB, sAB, Exp)
                else:
                    nc.scalar.activation(eAB[:, 0:L], sAB[:, 0:L], Exp)
                    nc.scalar.activation(eAB[:, SST:SST + L],
                                         sAB[:, SST:SST + L], Exp)
                if CHUNK * j >= SST * ks:  # diagonal tile: zero upper triangle
                    for base in (0, SST):
                        nc.gpsimd.affine_select(
                            out=eAB[:, base:base + CHUNK],
                            in_=eAB[:, base:base + CHUNK],
                            pattern=[[1, CHUNK]], channel_multiplier=-1,
                            base=0, compare_op=mybir.AluOpType.is_ge, fill=0.0)
                av = av_cur[0]
                nc.tensor.matmul(
                    av[0:VW, off:SST],
                    vaug[j][:, hA * VW:(hA + 1) * VW],
                    eAB[:, 0:L],
                    start=(j == 0), stop=(j == nj - 1), skip_group_check=True)
                nc.tensor.matmul(
                    av[0:VW, SST + off:2 * SST],
                    vaug[j][:, hB * VW:(hB + 1) * VW],
                    eAB[:, SST:SST + L],
                    start=(j == 0), stop=(j == nj - 1), skip_group_check=True)

            def normalize(p, ks):
                """softmax-normalize strip ks of pair p into aoT[p]."""
                av = av_cur[0]
                strip = slice(ks * SST, (ks + 1) * SST)
                den = wp.tile([1, 2 * SST], F32, tag="den", bufs=2,
                              name=f"den_{p}_{ks}")
                # evict (unnormalized) + denominator row; frees the av banks
                if ks < nks - 1 or p == 1:
                    nc.scalar.activation(den, av[D:D + 1, :],
                                         mybir.ActivationFunctionType.Copy)
                else:
                    nc.vector.tensor_copy(den, av[D:D + 1, :])
                nc.vector.tensor_copy(aoT[p][0:D, strip], av[0:D, 0:SST])
                nc.vector.tensor_copy(aoT[p][D:CHUNK, strip],
                                      av[0:D, SST:2 * SST])
                # reciprocal via a [128, 8] reshape (wide on the DVE lanes),
                # then partition-broadcast — both through small DRAM bounces
                # (SBUF APs cannot reshape across partitions / stride-0 bcast)
                dden = dr.tile([1, 2 * SST], F32, tag="dden", bufs=2,
                               name=f"dden_{p}_{ks}")
                nc.gpsimd.dma_start(out=dden, in_=den)
                nf = 2 * SST // CHUNK
                dsb = wp.tile([CHUNK, nf], F32, tag="dsb", bufs=2,
                              name=f"dsb_{p}_{ks}")
                dview = dden.rearrange("a b -> (a b)").rearrange(
                    "(p f) -> p f", p=CHUNK)
                nc.gpsimd.dma_start(out=dsb, in_=dview)
                rsb = wp.tile([CHUNK, nf], F32, tag="rsb", bufs=2,
                              name=f"rsb_{p}_{ks}")
                nc.vector.reciprocal(rsb, dsb)
                drec = dr.tile([1, 2 * SST], F32, tag="drec", bufs=2,
                               name=f"drec_{p}_{ks}")
                rview = drec.rearrange("a b -> (a b)").rearrange(
                    "(p f) -> p f", p=CHUNK)
                nc.gpsimd.dma_start(out=rview, in_=rsb)
                bcf = wp.tile([CHUNK, SST], F32, tag="bcf", bufs=2,
                              name=f"bcf_{p}_{ks}")
                nc.gpsimd.dma_start(
                    out=bcf[0:D, :],
                    in_=drec[0:1, 0:SST].broadcast_to((D, SST)))
                nc.gpsimd.dma_start(
                    out=bcf[D:CHUNK, :],
                    in_=drec[0:1, SST:2 * SST].broadcast_to((D, SST)))
                nc.gpsimd.tensor_mul(aoT[p][:, strip], aoT[p][:, strip], bcf)

            # =========== schedule ===========
            # pre-loop: full projection of strip 0
            for pj in (1, 0):
                for m in range(2):
                    proj_qk_group(0, pj, m, xt_bufs[0])
            for u in range(4):
                proj_v_group(0, u, xt_bufs[0])

            av_cur = [None]
            for ks in range(nks):
                # prefetch x strip ks+2 (its buffer was freed by proj(ks))
                if ks + 2 < nks:
                    xt_bufs[ks + 2] = load_xt(ks + 2, [nc.sync])

                fillers = []
                if ks + 1 < nks:                  # projection of next strip
                    xt = xt_bufs[ks + 1]
                    for pj in (1, 0):
                        for m in range(2):
                            fillers.append((proj_qk_group, (ks + 1, pj, m, xt)))
                    for u in range(4):
                        fillers.append((proj_v_group, (ks + 1, u, xt)))
                if ks == nks - 1:                 # deferred out-proj (strips 0..ks-1)
                    for jt in range(8):
                        fillers.append((outproj_group, (jt,)))

                nj = 4 * ks + 4
                total_js = 2 * nj
                gi = 0
                cnt = 0
                for p in (0, 1):
                    av_cur[0] = ps.tile([CHUNK, 2 * SST], F32, tag="av", bufs=1,
                                        name=f"av_{p}_{ks}")
                    for j in range(nj):
                        jstep(p, ks, j, nj)
                        cnt += 1
                        want = (cnt * len(fillers)) // total_js
                        while gi < want:
                            fn, args = fillers[gi]
                            fn(*args)
                            gi += 1
                    normalize(p, ks)
                    # keep the tensor queue fed while av drains
                    if gi < len(fillers):
                        fn, args = fillers[gi]
                        fn(*args)
                        gi += 1
                while gi < len(fillers):
                    fn, args = fillers[gi]
                    fn(*args)
                    gi += 1
                if ks == nks - 1:
                    # reserved groups overlap the final normalize chain
                    for jt in range(8, 4 * (nks - 1)):
                        outproj_group(jt, tail=True)

            # tail: out-proj of the last strip
            for jt in range(4 * (nks - 1), 4 * nks):
                outproj_group(jt, tail=True)

    if fix_waits:
        split_excess_waits(nc)
    return nc


def make_in_maps(x, wq, bq, wk, bk, wv, bv, wo, bo, t_len=T):
    """Build the 8 per-core input dicts from full inputs."""
    in_maps = []
    scale = 1.0 / np.sqrt(np.float32(D))
    for core in range(NCORES):
        b, hg = core // 4, core % 4
        sl = slice(DPC * hg, DPC * (hg + 1))
        wqs = (wq[sl] * scale).astype(np.float32)
        bqs = (bq[sl] * scale).astype(np.float32)
        wqkvT = np.concatenate([wqs.T, wk[sl].T, wv[sl].T], axis=1)
        bqk = np.stack([bqs[0:CHUNK], bqs[CHUNK:2 * CHUNK],
                        bk[sl][0:CHUNK], bk[sl][CHUNK:2 * CHUNK]], axis=1)
        in_maps.append({
            "xT": np.ascontiguousarray(x[b, :t_len].T).astype(BF16_NP),
            "wqkvT": np.ascontiguousarray(wqkvT).astype(BF16_NP),
            "woT": np.ascontiguousarray(wo[:, sl].T).astype(BF16_NP),
            "bqk": np.ascontiguousarray(bqk, dtype=np.float32),
            "bv_row": np.ascontiguousarray(bv[sl][None, :], dtype=np.float32),
        })
    return in_maps


def gather_output(results, bo, t_len=T):
    ys = [results[i]["y"] for i in range(NCORES)]
    out = np.stack([ys[0] + ys[1] + ys[2] + ys[3],
                    ys[4] + ys[5] + ys[6] + ys[7]]).astype(np.float32)
    out += np.asarray(bo, np.float32)[None, None, :]
    return out


_NC_CACHE = {}


def _get_nc(t_len=T):
    if t_len not in _NC_CACHE:
        _NC_CACHE[t_len] = build_nc(t_len)
    return _NC_CACHE[t_len]


def kernel(x, wq, bq, wk, bk, wv, bv, wo, bo, mask=None, **_unused):
    """Full-input entry point: shard, run on 8 NeuronCores, gather."""
    from concourse.bass_utils import run_bass_kernel_spmd

    x = np.asarray(x, dtype=np.float32)
    in_maps = make_in_maps(x, np.asarray(wq, np.float32), np.asarray(bq, np.float32),
                           np.asarray(wk, np.float32), np.asarray(bk, np.float32),
                           np.asarray(wv, np.float32), np.asarray(bv, np.float32),
                           np.asarray(wo, np.float32), np.asarray(bo, np.float32))
    nc = _get_nc(T)
    res = run_bass_kernel_spmd(nc, in_maps, list(range(NCORES)))
    return gather_output(res.results, bo)
